# revision 34
# baseline (speedup 1.0000x reference)
"""Chamfer-distance kernel for TRN2 (8 NeuronCores, SPMD).

Math: the reference weights w are nonzero ONLY for points with
time_indice == 1 (m of N points).  So of the NxN distance matrix we only
need row-mins for the m selected rows (dist1) and col-mins for the m
selected columns (dist2) -- each an (m x N) problem, min over N.

Each (m x N) pass is a K=11 fp16 matmul computing
    C[i, j] = sq[j] - 2 * dot(sel_i, pts_j)
EXACTLY (fp32-equivalent) via two-term fp16 splitting: with a = -2*sel
split as a_hi + a_lo and p split as p_hi + p_lo (fp16 hi/lo pairs),
    a.p = a_hi.p_hi + a_hi.p_lo + a_lo.p_hi   (+ a_lo.p_lo ~ 2^-22, dropped)
and sq[j] = sq_hi[j] + sq_lo[j] the same way.  Every fp16*fp16 product is
exact in fp32, so PSUM accumulates the fp32 result at fp16 single-pass
speed (1 col/cycle vs 4 for fp32's LOW/HIGH dual pass).  The per-row
constant sq[i] of the selected point is added on the host after the min.

Perf structure (per 128-row tile, 2048 columns on each core):
  * the 4 512-col chunk matmuls are packed into the 4 distinct PE
    row-groups via tile_position (K=11 occupies 11 of each group's 32
    rows), so they run concurrently: ~1 x 512 cycles per tile;
  * PSUM is split into lo/hi 2-bank tiles; the Scalar engine copies the
    hi half to SBUF while the Vector engine runs a runtime-registered
    custom DVE op (min2-reduce: out = min(in0,in1), accum_out = row-min)
    ingesting the PSUM lo half and the SBUF copy at 2 elements/cycle.
    DVE is the pacing engine at ~1024 cycles (~1.15us) per tile with
    zero inter-op idle; Scalar trails at ~1.12us/tile.  This is the
    hard floor: PSUM egress is limited to ACT 1 elem/cyc + DVE-in0
    1 elem/cyc, and no other engine can reach PSUM (gpsimd has no
    PSUM port, DMA is ~20x too slow for bulk PSUM reads).

Sharding: the N search points are split 2048-per-core across 8 cores
(same lhsT everywhere); each core returns per-row partial mins, the host
takes the elementwise min across cores and does the tiny O(m) tail.
"""

import numpy as np

import concourse.bass as bass
import concourse.mybir as mybir
import concourse.tile as tile
from concourse import bacc
from concourse import dve_ops as _dvo
from concourse.bass_utils import run_bass_kernel_spmd
from concourse.dve_spec import Spec, Src0, Src1, C0, AluOp, minn, lower
from concourse.dve_spec import _has_src1 as _has_src1
from concourse.dve_uop import DveOpSpec


def _make_min2():
    """Register a custom DVE op: out = min(in0, in1), accum_out = row-min.

    One output/cycle while ingesting TWO streams (PSUM + SBUF).  The stock
    InstTensorTensorReduce crashes the NEFF at runtime on this stack, so
    the baseline's runtime-registered custom op is used instead.
    """
    name = "MIN2_REDUCE_ANT"
    for o in _dvo.OPS:
        if o.name == name:
            return o

    def _ref(in0, in1, s0, s1, imm2):
        b = np.minimum(in0, in1).astype(np.float32)
        seed = np.asarray(s0, np.float32).reshape(-1, 1)
        acc = np.minimum(b.reshape(b.shape[0], -1).min(axis=-1, keepdims=True), seed)
        return b, acc

    spec = Spec(body=minn(Src0, Src1), accum=AluOp.MIN, accum_init=C0,
                reference=_ref)
    op = _dvo.DveOp(name, spec, subdim=False, uops_sha={})
    _dvo.OPS.append(op)
    _dvo.CUSTOM_DVE_SPECS[name] = spec
    _dvo._SUB_OPCODE_FOR_NAME[name] = _dvo._CUSTOM_DVE_ROW_BASE + len(_dvo.OPS) - 1
    for ver in ("v3", "v4"):
        ds = DveOpSpec(name=name, opcode=_dvo.get_dve_sub_opcode(name),
                       uops=lower(spec, ver=ver), rd1_en=_has_src1(spec))
        op.uops_sha[ver] = ds.sha(ver)
    return op


_MIN2 = _make_min2()

N_CORES = 8
N_POINTS = 16384
NSHARD = N_POINTS // N_CORES  # 2048 search points per core
FREE = 512                    # matmul moving free dim (one PSUM bank of fp32)
K = 11                        # 3x a_hi.p_hi + 3x a_hi.p_lo + 3x a_lo.p_hi + sq_hi + sq_lo

_CACHE = {}


def _build(n_rt):
    """Build + compile the SPMD Bass program for n_rt row-tiles of 128."""
    f32 = mybir.dt.float32
    f16 = mybir.dt.float16
    mpad = n_rt * 128
    ncc = NSHARD // FREE      # 4 column chunks of 512
    half = NSHARD // 2        # 1024

    nc = bacc.Bacc("TRN2", target_bir_lowering=False, debug=False,
                   num_devices=N_CORES, enable_partition_id=False)
    # rhs tensors carry the tile-0 lhs slice appended after the 512 rhs
    # columns, so ONE DMA per row-group delivers everything the first
    # row-tile's matmuls need (halves the critical head DMA chain).
    lhsA = nc.dram_tensor("lhsA", [ncc, K, mpad - 128], f16, kind="ExternalInput").ap()
    rhsA = nc.dram_tensor("rhsA", [ncc, K, FREE + 128], f16, kind="ExternalInput").ap()
    lhsB = nc.dram_tensor("lhsB", [ncc, K, mpad - 128], f16, kind="ExternalInput").ap()
    rhsB = nc.dram_tensor("rhsB", [ncc, K, FREE + 128], f16, kind="ExternalInput").ap()
    outA = nc.dram_tensor("outA", [128, n_rt], f32, kind="ExternalOutput").ap()
    outB = nc.dram_tensor("outB", [128, n_rt], f32, kind="ExternalOutput").ap()

    with tile.TileContext(nc) as tc:
        with (
            tc.tile_pool(name="inp", bufs=1) as inp,
            tc.tile_pool(name="res", bufs=1) as res,
            tc.tile_pool(name="cpy", bufs=4) as cpy,
            tc.tile_pool(name="scr", bufs=4) as scr,
            tc.tile_pool(name="pslo", bufs=2, space="PSUM") as pslo,
            tc.tile_pool(name="pshi", bufs=2, space="PSUM") as pshi,
        ):
            # Row-group g's [K, w] slab lives at partitions 32g..32g+K-1.
            # lA/lB hold row-tiles 1..n_rt-1 (tile 0 rides in rA/rB).
            lA = inp.tile([128, mpad - 128], f16, tag="lA")
            rA = inp.tile([128, FREE + 128], f16, tag="rA")
            lB = inp.tile([128, mpad - 128], f16, tag="lB")
            rB = inp.tile([128, FREE + 128], f16, tag="rB")

            # The head is DMA-latency bound: each queue serializes issue
            # (~0.76us) + sem-prop (~0.9us) per DMA, and per-queue transfers
            # serialize too.  Spread the pass-A groups across the three
            # legal DMA queues (SP/sync, Activation/scalar, gpsimd).
            # Pass-B inputs queue behind (needed ~25us in).
            # Queue plan (measured): DMA issue serializes with compute on
            # the issuing sequencer, so the scalar queue gets only two lhs
            # slabs whose issues finish before the first PSUM->SBUF copy;
            # gpsimd's SWDGE is slow (~1.6us end-to-end each, serial) so it
            # gets one critical rhs chunk + the late pass-B inputs; sync
            # carries the rest.  Copy-gating chunks (0,2 -> pt_hi) ride the
            # fastest slots.
            gp = [slice(32 * b, 32 * b + K) for b in range(ncc)]
            nc.sync.dma_start(out=rA[gp[0], :], in_=rhsA[0])
            nc.sync.dma_start(out=rA[gp[2], :], in_=rhsA[2])
            nc.gpsimd.dma_start(out=rA[gp[1], :], in_=rhsA[1])
            nc.sync.dma_start(out=rA[gp[3], :], in_=rhsA[3])
            # sync's lhs slabs are split so the tiles-1..6 halves land
            # before row-tile 1 needs them (behind three rhs transfers).
            hsp = (mpad - 128) // 2
            nc.sync.dma_start(out=lA[gp[0], 0:hsp], in_=lhsA[0][:, 0:hsp])
            nc.sync.dma_start(out=lA[gp[3], 0:hsp], in_=lhsA[3][:, 0:hsp])
            nc.scalar.dma_start(out=lA[gp[1], :], in_=lhsA[1])
            nc.scalar.dma_start(out=lA[gp[2], :], in_=lhsA[2])
            nc.sync.dma_start(out=lA[gp[0], hsp:], in_=lhsA[0][:, hsp:])
            nc.sync.dma_start(out=lA[gp[3], hsp:], in_=lhsA[3][:, hsp:])
            for b in range(ncc):
                nc.sync.dma_start(out=lB[gp[b], :], in_=lhsB[b])
                nc.gpsimd.dma_start(out=rB[gp[b], :], in_=rhsB[b])

            mA = res.tile([128, n_rt], f32, tag="mA")
            mB = res.tile([128, n_rt], f32, tag="mB")

            for pi, (lhs, rhs, mins) in enumerate(((lA, rA, mA), (lB, rB, mB))):
                for rt in range(n_rt):
                    pt_lo = pslo.tile([128, half], f32, tag="pslo")
                    pt_hi = pshi.tile([128, half], f32, tag="pshi")
                    for cc in range(ncc):
                        # early-arriving chunks go to pt_hi (ACT-copied, the
                        # copy runs while later chunks compute); the last
                        # chunks land in pt_lo which the DVE reads directly.
                        dst = pt_hi if cc % 2 == 0 else pt_lo
                        dsl = dst[:, bass.ts(cc // 2, FREE)]
                        p = slice(32 * cc, 32 * cc + K)
                        lhsT = (rhs[p, FREE:FREE + 128] if rt == 0
                                else lhs[p, bass.ts(rt - 1, 128)])
                        nc.tensor.matmul(
                            dsl,
                            lhsT,
                            rhs[p, 0:FREE],
                            start=True, stop=True,
                            tile_position=(32 * cc, 0),
                        )
                    # ACT copies the upper PSUM half to SBUF; DVE custom
                    # min2-reduce folds the lower PSUM half against it while
                    # row-min-reducing into mins[:, rt].
                    cp = cpy.tile([128, half], f32, tag="cp")
                    sc = scr.tile([128, half], f32, tag="sc")
                    nc.scalar.copy(out=cp[:], in_=pt_hi[:, :])
                    nc.vector._custom_dve(
                        _MIN2, out=sc[:], in0=pt_lo[:, :], in1=cp[:],
                        s0=3.0e38, accum_out=mins[:, rt:rt + 1])

            nc.sync.dma_start(out=outA, in_=mA[:])
            nc.sync.dma_start(out=outB, in_=mB[:])

    nc.compile()
    return nc


def _get_program(n_rt):
    if n_rt not in _CACHE:
        _CACHE[n_rt] = _build(n_rt)
    return _CACHE[n_rt]


def _transform(points, poses, idx):
    P = poses[idx]                                   # [N,4,4]
    R, t = P[:, :3, :3], P[:, :3, 3]
    return np.einsum('nij,nj->ni', R, points) + t    # [N,3]


def _split16(x):
    """Two-term fp16 split: x ~= hi + lo with hi = fp16(x)."""
    hi = x.astype(np.float16)
    lo = (x - hi.astype(np.float64)).astype(np.float16)
    return hi, lo


def kernel(points, time_indice, est_poses, gt_poses):
    points = np.asarray(points, dtype=np.float32)
    ti = np.asarray(time_indice)
    est_poses = np.asarray(est_poses, dtype=np.float32)
    gt_poses = np.asarray(gt_poses, dtype=np.float32)

    est = _transform(points, est_poses, ti).astype(np.float64)  # [N,3]
    gt = _transform(points, gt_poses, ti).astype(np.float64)    # [N,3]
    est_sq = np.sum(est * est, axis=1)               # [N] f64
    gt_sq = np.sum(gt * gt, axis=1)                  # [N] f64

    sel = np.flatnonzero(ti == 1)
    m = sel.size
    denom = np.float32(m) + np.float32(1e-7)
    if m == 0:
        return np.float32(0.0), np.float32(0.0)

    l2 = np.float32(
        np.linalg.norm(est[sel] - gt[sel], axis=1).sum() / denom)

    n_rt = -(-m // 128)
    mpad = n_rt * 128
    pad = np.concatenate([sel, np.repeat(sel[:1], mpad - m)])
    ncc = NSHARD // FREE

    def lhs_rows(sel_pts):
        a = -2.0 * sel_pts[pad]                      # [mpad,3] f64
        a_hi, a_lo = _split16(a)
        out = np.empty((K, mpad), np.float16)
        out[0:3] = a_hi.T
        out[3:6] = a_hi.T
        out[6:9] = a_lo.T
        out[9:11] = np.float16(1.0)
        return out

    def lhs_for(lrows):
        # row-tiles 1.. only; tile 0 is packed into the rhs tensor
        return np.ascontiguousarray(
            np.broadcast_to(lrows[:, 128:], (ncc, K, mpad - 128)))

    def rhs_for(pts, sq, c, lrows):
        s = slice(c * NSHARD, (c + 1) * NSHARD)
        p_hi, p_lo = _split16(pts[s])                # [2048,3]
        q_hi, q_lo = _split16(sq[s])                 # [2048]
        out = np.empty((K, NSHARD), np.float16)
        out[0:3] = p_hi.T
        out[3:6] = p_lo.T
        out[6:9] = p_hi.T
        out[9] = q_hi
        out[10] = q_lo
        # [K, 2048] -> [4, K, 512] chunk-major, with the tile-0 lhs slice
        # appended to each group so one DMA delivers both
        chunks = out.reshape(K, ncc, FREE).transpose(1, 0, 2)
        l0 = np.broadcast_to(lrows[:, 0:128], (ncc, K, 128))
        return np.ascontiguousarray(np.concatenate([chunks, l0], axis=2))

    lrowsA = lhs_rows(gt)   # dist1: selected gt rows vs all est points
    lrowsB = lhs_rows(est)  # dist2: selected est rows vs all gt points
    lhsA = lhs_for(lrowsA)
    lhsB = lhs_for(lrowsB)
    in_maps = [
        {
            "lhsA": lhsA,
            "rhsA": rhs_for(est, est_sq, c, lrowsA),
            "lhsB": lhsB,
            "rhsB": rhs_for(gt, gt_sq, c, lrowsB),
        }
        for c in range(N_CORES)
    ]

    nc = _get_program(n_rt)
    results = run_bass_kernel_spmd(nc, in_maps, list(range(N_CORES))).results

    # [128, n_rt] per core -> global min across cores -> flatten row-tiles
    partA = np.min([r["outA"] for r in results], axis=0).T.ravel()[:m]
    partB = np.min([r["outB"] for r in results], axis=0).T.ravel()[:m]
    dist1 = partA.astype(np.float64) + gt_sq[sel]
    dist2 = partB.astype(np.float64) + est_sq[sel]
    chamfer = np.float32(0.5 * (dist1.sum() + dist2.sum()) / denom)
    return chamfer, l2


# revision 35
# speedup vs baseline: 1.1707x; 1.1707x over previous
"""Chamfer-distance kernel for TRN2 (8 NeuronCores, SPMD).

Math: the reference weights w are nonzero ONLY for points with
time_indice == 1 (m of N points).  So of the NxN distance matrix we only
need row-mins for the m selected rows (dist1) and col-mins for the m
selected columns (dist2) -- each an (m x N) problem, min over N.

Each (m x N) pass is a K=11 fp16 matmul computing
    C[i, j] = sq[j] - 2 * dot(sel_i, pts_j)
EXACTLY (fp32-equivalent) via two-term fp16 splitting: with a = -2*sel
split as a_hi + a_lo and p split as p_hi + p_lo (fp16 hi/lo pairs),
    a.p = a_hi.p_hi + a_hi.p_lo + a_lo.p_hi   (+ a_lo.p_lo ~ 2^-22, dropped)
and sq[j] = sq_hi[j] + sq_lo[j] the same way.  Every fp16*fp16 product is
exact in fp32, so PSUM accumulates the fp32 result at fp16 single-pass
speed (1 col/cycle vs 4 for fp32's LOW/HIGH dual pass).  The per-row
constant sq[i] of the selected point is added on the host after the min.

Perf structure (per 128-row tile, 2048 columns on each core):
  * the 4 512-col chunk matmuls are packed into the 4 distinct PE
    row-groups via tile_position (K=11 occupies 11 of each group's 32
    rows), so they run concurrently: ~1 x 512 cycles per tile;
  * PSUM is split into lo/hi 2-bank tiles; the Scalar engine copies the
    hi half to SBUF while the Vector engine runs a runtime-registered
    custom DVE op (min2-reduce: out = min(in0,in1), accum_out = row-min)
    ingesting the PSUM lo half and the SBUF copy at 2 elements/cycle.
    DVE is the pacing engine at ~1024 cycles (~1.15us) per tile with
    zero inter-op idle; Scalar trails at ~1.12us/tile.  This is the
    hard floor: PSUM egress is limited to ACT 1 elem/cyc + DVE-in0
    1 elem/cyc, and no other engine can reach PSUM (gpsimd has no
    PSUM port, DMA is ~20x too slow for bulk PSUM reads).

Sharding: the N search points are split 2048-per-core across 8 cores
(same lhsT everywhere); each core returns per-row partial mins, the host
takes the elementwise min across cores and does the tiny O(m) tail.
"""

import numpy as np

import concourse.bass as bass
import concourse.mybir as mybir
import concourse.tile as tile
from concourse import bacc
from concourse import dve_ops as _dvo
from concourse.bass_utils import run_bass_kernel_spmd
from concourse.dve_spec import Spec, Src0, Src1, C0, AluOp, minn, lower
from concourse.dve_spec import _has_src1 as _has_src1
from concourse.dve_uop import DveOpSpec


def _make_min2():
    """Register a custom DVE op: out = min(in0, in1), accum_out = row-min.

    One output/cycle while ingesting TWO streams (PSUM + SBUF).  The stock
    InstTensorTensorReduce crashes the NEFF at runtime on this stack, so
    the baseline's runtime-registered custom op is used instead.
    """
    name = "MIN2_REDUCE_ANT"
    for o in _dvo.OPS:
        if o.name == name:
            return o

    def _ref(in0, in1, s0, s1, imm2):
        b = np.minimum(in0, in1).astype(np.float32)
        seed = np.asarray(s0, np.float32).reshape(-1, 1)
        acc = np.minimum(b.reshape(b.shape[0], -1).min(axis=-1, keepdims=True), seed)
        return b, acc

    spec = Spec(body=minn(Src0, Src1), accum=AluOp.MIN, accum_init=C0,
                reference=_ref)
    op = _dvo.DveOp(name, spec, subdim=False, uops_sha={})
    _dvo.OPS.append(op)
    _dvo.CUSTOM_DVE_SPECS[name] = spec
    _dvo._SUB_OPCODE_FOR_NAME[name] = _dvo._CUSTOM_DVE_ROW_BASE + len(_dvo.OPS) - 1
    for ver in ("v3", "v4"):
        ds = DveOpSpec(name=name, opcode=_dvo.get_dve_sub_opcode(name),
                       uops=lower(spec, ver=ver), rd1_en=_has_src1(spec))
        op.uops_sha[ver] = ds.sha(ver)
    return op


_MIN2 = _make_min2()

N_CORES = 8
N_POINTS = 16384
NSHARD = N_POINTS // N_CORES  # 2048 search points per core
FREE = 512                    # matmul moving free dim (one PSUM bank of fp32)
K = 11                        # 3x a_hi.p_hi + 3x a_hi.p_lo + 3x a_lo.p_hi + sq_hi + sq_lo

_CACHE = {}


def _build(n_rt):
    """Build + compile the SPMD Bass program for n_rt row-tiles of 128."""
    f32 = mybir.dt.float32
    f16 = mybir.dt.float16
    mpad = n_rt * 128
    ncc = NSHARD // FREE      # 4 column chunks of 512
    half = NSHARD // 2        # 1024

    nc = bacc.Bacc("TRN2", target_bir_lowering=False, debug=False,
                   num_devices=N_CORES, enable_partition_id=False)
    # rhs tensors carry the tile-0 lhs slice appended after the 512 rhs
    # columns, so ONE DMA per row-group delivers everything the first
    # row-tile's matmuls need (halves the critical head DMA chain).
    lhsA = nc.dram_tensor("lhsA", [ncc, K, mpad - 128], f16, kind="ExternalInput").ap()
    rhsA = nc.dram_tensor("rhsA", [ncc, K, FREE + 128], f16, kind="ExternalInput").ap()
    lhsB = nc.dram_tensor("lhsB", [ncc, K, mpad - 128], f16, kind="ExternalInput").ap()
    rhsB = nc.dram_tensor("rhsB", [ncc, K, FREE + 128], f16, kind="ExternalInput").ap()
    outA = nc.dram_tensor("outA", [128, n_rt], f32, kind="ExternalOutput").ap()
    outB = nc.dram_tensor("outB", [128, n_rt], f32, kind="ExternalOutput").ap()

    with tile.TileContext(nc) as tc:
        with (
            tc.tile_pool(name="inp", bufs=1) as inp,
            tc.tile_pool(name="res", bufs=1) as res,
            tc.tile_pool(name="cpy", bufs=4) as cpy,
            tc.tile_pool(name="scr", bufs=4) as scr,
            tc.tile_pool(name="pslo", bufs=2, space="PSUM") as pslo,
            tc.tile_pool(name="pshi", bufs=2, space="PSUM") as pshi,
        ):
            # Row-group g's [K, w] slab lives at partitions 32g..32g+K-1.
            # lA/lB hold row-tiles 1..n_rt-1 (tile 0 rides in rA/rB).
            lA = inp.tile([128, mpad - 128], f16, tag="lA")
            rA = inp.tile([128, FREE + 128], f16, tag="rA")
            lB = inp.tile([128, mpad - 128], f16, tag="lB")
            rB = inp.tile([128, FREE + 128], f16, tag="rB")

            # The head is DMA-latency bound: each queue serializes issue
            # (~0.76us) + sem-prop (~0.9us) per DMA, and per-queue transfers
            # serialize too.  Spread the pass-A groups across the three
            # legal DMA queues (SP/sync, Activation/scalar, gpsimd).
            # Pass-B inputs queue behind (needed ~25us in).
            # Queue plan (measured): DMA issue serializes with compute on
            # the issuing sequencer, so the scalar queue gets only two lhs
            # slabs whose issues finish before the first PSUM->SBUF copy;
            # gpsimd's SWDGE is slow (~1.6us end-to-end each, serial) so it
            # gets one critical rhs chunk + the late pass-B inputs; sync
            # carries the rest.  Copy-gating chunks (0,2 -> pt_hi) ride the
            # fastest slots.
            gp = [slice(32 * b, 32 * b + K) for b in range(ncc)]
            nc.sync.dma_start(out=rA[gp[0], :], in_=rhsA[0])
            nc.sync.dma_start(out=rA[gp[2], :], in_=rhsA[2])
            nc.gpsimd.dma_start(out=rA[gp[1], :], in_=rhsA[1])
            nc.scalar.dma_start(out=rA[gp[3], :], in_=rhsA[3])
            # sync's lhs slabs are split so the tiles-1..6 halves land
            # before row-tile 1 needs them (behind three rhs transfers).
            hsp = (mpad - 128) // 2
            nc.sync.dma_start(out=lA[gp[0], 0:hsp], in_=lhsA[0][:, 0:hsp])
            nc.sync.dma_start(out=lA[gp[3], 0:hsp], in_=lhsA[3][:, 0:hsp])
            nc.scalar.dma_start(out=lA[gp[1], :], in_=lhsA[1])
            nc.scalar.dma_start(out=lA[gp[2], :], in_=lhsA[2])
            nc.sync.dma_start(out=lA[gp[0], hsp:], in_=lhsA[0][:, hsp:])
            nc.sync.dma_start(out=lA[gp[3], hsp:], in_=lhsA[3][:, hsp:])
            for b in range(ncc):
                nc.sync.dma_start(out=lB[gp[b], :], in_=lhsB[b])
                nc.gpsimd.dma_start(out=rB[gp[b], :], in_=rhsB[b])

            mA = res.tile([128, n_rt], f32, tag="mA")
            mB = res.tile([128, n_rt], f32, tag="mB")

            for pi, (lhs, rhs, mins) in enumerate(((lA, rA, mA), (lB, rB, mB))):
                for rt in range(n_rt):
                    pt_lo = pslo.tile([128, half], f32, tag="pslo")
                    pt_hi = pshi.tile([128, half], f32, tag="pshi")
                    for cc in range(ncc):
                        # early-arriving chunks go to pt_hi (ACT-copied, the
                        # copy runs while later chunks compute); the last
                        # chunks land in pt_lo which the DVE reads directly.
                        dst = pt_hi if cc % 2 == 0 else pt_lo
                        dsl = dst[:, bass.ts(cc // 2, FREE)]
                        p = slice(32 * cc, 32 * cc + K)
                        lhsT = (rhs[p, FREE:FREE + 128] if rt == 0
                                else lhs[p, bass.ts(rt - 1, 128)])
                        nc.tensor.matmul(
                            dsl,
                            lhsT,
                            rhs[p, 0:FREE],
                            start=True, stop=True,
                            tile_position=(32 * cc, 0),
                        )
                    # ACT copies the upper PSUM half to SBUF; DVE custom
                    # min2-reduce folds the lower PSUM half against it while
                    # row-min-reducing into mins[:, rt].
                    cp = cpy.tile([128, half], f32, tag="cp")
                    sc = scr.tile([128, half], f32, tag="sc")
                    nc.scalar.copy(out=cp[:], in_=pt_hi[:, :])
                    nc.vector._custom_dve(
                        _MIN2, out=sc[:], in0=pt_lo[:, :], in1=cp[:],
                        s0=3.0e38, accum_out=mins[:, rt:rt + 1])

            nc.sync.dma_start(out=outA, in_=mA[:])
            nc.sync.dma_start(out=outB, in_=mB[:])

    nc.compile()
    return nc


def _get_program(n_rt):
    if n_rt not in _CACHE:
        _CACHE[n_rt] = _build(n_rt)
    return _CACHE[n_rt]


def _transform(points, poses, idx):
    P = poses[idx]                                   # [N,4,4]
    R, t = P[:, :3, :3], P[:, :3, 3]
    return np.einsum('nij,nj->ni', R, points) + t    # [N,3]


def _split16(x):
    """Two-term fp16 split: x ~= hi + lo with hi = fp16(x)."""
    hi = x.astype(np.float16)
    lo = (x - hi.astype(np.float64)).astype(np.float16)
    return hi, lo


def kernel(points, time_indice, est_poses, gt_poses):
    points = np.asarray(points, dtype=np.float32)
    ti = np.asarray(time_indice)
    est_poses = np.asarray(est_poses, dtype=np.float32)
    gt_poses = np.asarray(gt_poses, dtype=np.float32)

    est = _transform(points, est_poses, ti).astype(np.float64)  # [N,3]
    gt = _transform(points, gt_poses, ti).astype(np.float64)    # [N,3]
    est_sq = np.sum(est * est, axis=1)               # [N] f64
    gt_sq = np.sum(gt * gt, axis=1)                  # [N] f64

    sel = np.flatnonzero(ti == 1)
    m = sel.size
    denom = np.float32(m) + np.float32(1e-7)
    if m == 0:
        return np.float32(0.0), np.float32(0.0)

    l2 = np.float32(
        np.linalg.norm(est[sel] - gt[sel], axis=1).sum() / denom)

    n_rt = -(-m // 128)
    mpad = n_rt * 128
    pad = np.concatenate([sel, np.repeat(sel[:1], mpad - m)])
    ncc = NSHARD // FREE

    def lhs_rows(sel_pts):
        a = -2.0 * sel_pts[pad]                      # [mpad,3] f64
        a_hi, a_lo = _split16(a)
        out = np.empty((K, mpad), np.float16)
        out[0:3] = a_hi.T
        out[3:6] = a_hi.T
        out[6:9] = a_lo.T
        out[9:11] = np.float16(1.0)
        return out

    def lhs_for(lrows):
        # row-tiles 1.. only; tile 0 is packed into the rhs tensor
        return np.ascontiguousarray(
            np.broadcast_to(lrows[:, 128:], (ncc, K, mpad - 128)))

    def rhs_for(pts, sq, c, lrows):
        s = slice(c * NSHARD, (c + 1) * NSHARD)
        p_hi, p_lo = _split16(pts[s])                # [2048,3]
        q_hi, q_lo = _split16(sq[s])                 # [2048]
        out = np.empty((K, NSHARD), np.float16)
        out[0:3] = p_hi.T
        out[3:6] = p_lo.T
        out[6:9] = p_hi.T
        out[9] = q_hi
        out[10] = q_lo
        # [K, 2048] -> [4, K, 512] chunk-major, with the tile-0 lhs slice
        # appended to each group so one DMA delivers both
        chunks = out.reshape(K, ncc, FREE).transpose(1, 0, 2)
        l0 = np.broadcast_to(lrows[:, 0:128], (ncc, K, 128))
        return np.ascontiguousarray(np.concatenate([chunks, l0], axis=2))

    lrowsA = lhs_rows(gt)   # dist1: selected gt rows vs all est points
    lrowsB = lhs_rows(est)  # dist2: selected est rows vs all gt points
    lhsA = lhs_for(lrowsA)
    lhsB = lhs_for(lrowsB)
    in_maps = [
        {
            "lhsA": lhsA,
            "rhsA": rhs_for(est, est_sq, c, lrowsA),
            "lhsB": lhsB,
            "rhsB": rhs_for(gt, gt_sq, c, lrowsB),
        }
        for c in range(N_CORES)
    ]

    nc = _get_program(n_rt)
    results = run_bass_kernel_spmd(nc, in_maps, list(range(N_CORES))).results

    # [128, n_rt] per core -> global min across cores -> flatten row-tiles
    partA = np.min([r["outA"] for r in results], axis=0).T.ravel()[:m]
    partB = np.min([r["outB"] for r in results], axis=0).T.ravel()[:m]
    dist1 = partA.astype(np.float64) + gt_sq[sel]
    dist2 = partB.astype(np.float64) + est_sq[sel]
    chamfer = np.float32(0.5 * (dist1.sum() + dist2.sum()) / denom)
    return chamfer, l2


# revision 36
# speedup vs baseline: 1.1751x; 1.0038x over previous
"""Chamfer-distance kernel for TRN2 (8 NeuronCores, SPMD).

Math: the reference weights w are nonzero ONLY for points with
time_indice == 1 (m of N points).  So of the NxN distance matrix we only
need row-mins for the m selected rows (dist1) and col-mins for the m
selected columns (dist2) -- each an (m x N) problem, min over N.

Each (m x N) pass is a K=11 fp16 matmul computing
    C[i, j] = sq[j] - 2 * dot(sel_i, pts_j)
EXACTLY (fp32-equivalent) via two-term fp16 splitting: with a = -2*sel
split as a_hi + a_lo and p split as p_hi + p_lo (fp16 hi/lo pairs),
    a.p = a_hi.p_hi + a_hi.p_lo + a_lo.p_hi   (+ a_lo.p_lo ~ 2^-22, dropped)
and sq[j] = sq_hi[j] + sq_lo[j] the same way.  Every fp16*fp16 product is
exact in fp32, so PSUM accumulates the fp32 result at fp16 single-pass
speed (1 col/cycle vs 4 for fp32's LOW/HIGH dual pass).  The per-row
constant sq[i] of the selected point is added on the host after the min.

Perf structure (per 128-row tile, 2048 columns on each core):
  * the 4 512-col chunk matmuls are packed into the 4 distinct PE
    row-groups via tile_position (K=11 occupies 11 of each group's 32
    rows), so they run concurrently: ~1 x 512 cycles per tile;
  * PSUM is split into lo/hi 2-bank tiles; the Scalar engine copies the
    hi half to SBUF while the Vector engine runs a runtime-registered
    custom DVE op (min2-reduce: out = min(in0,in1), accum_out = row-min)
    ingesting the PSUM lo half and the SBUF copy at 2 elements/cycle.
    DVE is the pacing engine at ~1024 cycles (~1.15us) per tile with
    zero inter-op idle; Scalar trails at ~1.12us/tile.  This is the
    hard floor: PSUM egress is limited to ACT 1 elem/cyc + DVE-in0
    1 elem/cyc, and no other engine can reach PSUM (gpsimd has no
    PSUM port, DMA is ~20x too slow for bulk PSUM reads).

Sharding: the N search points are split 2048-per-core across 8 cores
(same lhsT everywhere); each core returns per-row partial mins, the host
takes the elementwise min across cores and does the tiny O(m) tail.
"""

import numpy as np

import concourse.bass as bass
import concourse.mybir as mybir
import concourse.tile as tile
from concourse import bacc
from concourse import dve_ops as _dvo
from concourse.bass_utils import run_bass_kernel_spmd
from concourse.dve_spec import Spec, Src0, Src1, C0, AluOp, minn, lower
from concourse.dve_spec import _has_src1 as _has_src1
from concourse.dve_uop import DveOpSpec


def _make_min2():
    """Register a custom DVE op: out = min(in0, in1), accum_out = row-min.

    One output/cycle while ingesting TWO streams (PSUM + SBUF).  The stock
    InstTensorTensorReduce crashes the NEFF at runtime on this stack, so
    the baseline's runtime-registered custom op is used instead.
    """
    name = "MIN2_REDUCE_ANT"
    for o in _dvo.OPS:
        if o.name == name:
            return o

    def _ref(in0, in1, s0, s1, imm2):
        b = np.minimum(in0, in1).astype(np.float32)
        seed = np.asarray(s0, np.float32).reshape(-1, 1)
        acc = np.minimum(b.reshape(b.shape[0], -1).min(axis=-1, keepdims=True), seed)
        return b, acc

    spec = Spec(body=minn(Src0, Src1), accum=AluOp.MIN, accum_init=C0,
                reference=_ref)
    op = _dvo.DveOp(name, spec, subdim=False, uops_sha={})
    _dvo.OPS.append(op)
    _dvo.CUSTOM_DVE_SPECS[name] = spec
    _dvo._SUB_OPCODE_FOR_NAME[name] = _dvo._CUSTOM_DVE_ROW_BASE + len(_dvo.OPS) - 1
    for ver in ("v3", "v4"):
        ds = DveOpSpec(name=name, opcode=_dvo.get_dve_sub_opcode(name),
                       uops=lower(spec, ver=ver), rd1_en=_has_src1(spec))
        op.uops_sha[ver] = ds.sha(ver)
    return op


_MIN2 = _make_min2()

N_CORES = 8
N_POINTS = 16384
NSHARD = N_POINTS // N_CORES  # 2048 search points per core
FREE = 512                    # matmul moving free dim (one PSUM bank of fp32)
K = 11                        # 3x a_hi.p_hi + 3x a_hi.p_lo + 3x a_lo.p_hi + sq_hi + sq_lo

_CACHE = {}


def _build(n_rt):
    """Build + compile the SPMD Bass program for n_rt row-tiles of 128."""
    f32 = mybir.dt.float32
    f16 = mybir.dt.float16
    mpad = n_rt * 128
    ncc = NSHARD // FREE      # 4 column chunks of 512
    half = NSHARD // 2        # 1024

    nc = bacc.Bacc("TRN2", target_bir_lowering=False, debug=False,
                   num_devices=N_CORES, enable_partition_id=False)
    # rhs tensors carry the tile-0 lhs slice appended after the 512 rhs
    # columns, so ONE DMA per row-group delivers everything the first
    # row-tile's matmuls need (halves the critical head DMA chain).
    lhsA = nc.dram_tensor("lhsA", [ncc, K, mpad - 128], f16, kind="ExternalInput").ap()
    rhsA = nc.dram_tensor("rhsA", [ncc, K, FREE + 128], f16, kind="ExternalInput").ap()
    lhsB = nc.dram_tensor("lhsB", [ncc, K, mpad - 128], f16, kind="ExternalInput").ap()
    rhsB = nc.dram_tensor("rhsB", [ncc, K, FREE + 128], f16, kind="ExternalInput").ap()
    outA = nc.dram_tensor("outA", [128, n_rt], f32, kind="ExternalOutput").ap()
    outB = nc.dram_tensor("outB", [128, n_rt], f32, kind="ExternalOutput").ap()

    with tile.TileContext(nc) as tc:
        with (
            tc.tile_pool(name="inp", bufs=1) as inp,
            tc.tile_pool(name="res", bufs=1) as res,
            tc.tile_pool(name="cpy", bufs=4) as cpy,
            tc.tile_pool(name="scr", bufs=4) as scr,
            tc.tile_pool(name="pslo", bufs=2, space="PSUM") as pslo,
            tc.tile_pool(name="pshi", bufs=2, space="PSUM") as pshi,
        ):
            # Row-group g's [K, w] slab lives at partitions 32g..32g+K-1.
            # lA/lB hold row-tiles 1..n_rt-1 (tile 0 rides in rA/rB).
            lA = inp.tile([128, mpad - 128], f16, tag="lA")
            rA = inp.tile([128, FREE + 128], f16, tag="rA")
            lB = inp.tile([128, mpad - 128], f16, tag="lB")
            rB = inp.tile([128, FREE + 128], f16, tag="rB")

            # The head is DMA-latency bound: each queue serializes issue
            # (~0.76us) + sem-prop (~0.9us) per DMA, and per-queue transfers
            # serialize too.  Spread the pass-A groups across the three
            # legal DMA queues (SP/sync, Activation/scalar, gpsimd).
            # Pass-B inputs queue behind (needed ~25us in).
            # Queue plan (measured): DMA issue serializes with compute on
            # the issuing sequencer, so the scalar queue gets only two lhs
            # slabs whose issues finish before the first PSUM->SBUF copy;
            # gpsimd's SWDGE is slow (~1.6us end-to-end each, serial) so it
            # gets one critical rhs chunk + the late pass-B inputs; sync
            # carries the rest.  Copy-gating chunks (0,2 -> pt_hi) ride the
            # fastest slots.
            gp = [slice(32 * b, 32 * b + K) for b in range(ncc)]
            nc.sync.dma_start(out=rA[gp[0], :], in_=rhsA[0])
            nc.sync.dma_start(out=rA[gp[2], :], in_=rhsA[2])
            nc.gpsimd.dma_start(out=rA[gp[1], :], in_=rhsA[1])
            nc.sync.dma_start(out=rA[gp[3], :], in_=rhsA[3])
            # sync's lhs slabs are split so the tiles-1..6 halves land
            # before row-tile 1 needs them (behind three rhs transfers).
            hsp = (mpad - 128) // 2
            nc.sync.dma_start(out=lA[gp[0], 0:hsp], in_=lhsA[0][:, 0:hsp])
            nc.sync.dma_start(out=lA[gp[3], 0:hsp], in_=lhsA[3][:, 0:hsp])
            nc.scalar.dma_start(out=lA[gp[1], :], in_=lhsA[1])
            nc.scalar.dma_start(out=lA[gp[2], :], in_=lhsA[2])
            nc.sync.dma_start(out=lA[gp[0], hsp:], in_=lhsA[0][:, hsp:])
            nc.sync.dma_start(out=lA[gp[3], hsp:], in_=lhsA[3][:, hsp:])
            for b in range(ncc):
                nc.sync.dma_start(out=lB[gp[b], :], in_=lhsB[b])
                nc.gpsimd.dma_start(out=rB[gp[b], :], in_=rhsB[b])

            mA = res.tile([128, n_rt], f32, tag="mA")
            mB = res.tile([128, n_rt], f32, tag="mB")

            for pi, (lhs, rhs, mins) in enumerate(((lA, rA, mA), (lB, rB, mB))):
                for rt in range(n_rt):
                    pt_lo = pslo.tile([128, half], f32, tag="pslo")
                    pt_hi = pshi.tile([128, half], f32, tag="pshi")
                    for cc in range(ncc):
                        # early-arriving chunks go to pt_hi (ACT-copied, the
                        # copy runs while later chunks compute); the last
                        # chunks land in pt_lo which the DVE reads directly.
                        dst = pt_hi if cc % 2 == 0 else pt_lo
                        dsl = dst[:, bass.ts(cc // 2, FREE)]
                        p = slice(32 * cc, 32 * cc + K)
                        lhsT = (rhs[p, FREE:FREE + 128] if rt == 0
                                else lhs[p, bass.ts(rt - 1, 128)])
                        nc.tensor.matmul(
                            dsl,
                            lhsT,
                            rhs[p, 0:FREE],
                            start=True, stop=True,
                            tile_position=(32 * cc, 0),
                        )
                    # ACT copies the upper PSUM half to SBUF; DVE custom
                    # min2-reduce folds the lower PSUM half against it while
                    # row-min-reducing into mins[:, rt].
                    cp = cpy.tile([128, half], f32, tag="cp")
                    sc = scr.tile([128, half], f32, tag="sc")
                    nc.scalar.copy(out=cp[:], in_=pt_hi[:, :])
                    nc.vector._custom_dve(
                        _MIN2, out=sc[:], in0=pt_lo[:, :], in1=cp[:],
                        s0=3.0e38, accum_out=mins[:, rt:rt + 1])

            nc.sync.dma_start(out=outA, in_=mA[:])
            nc.sync.dma_start(out=outB, in_=mB[:])

    nc.compile()
    return nc


def _get_program(n_rt):
    if n_rt not in _CACHE:
        _CACHE[n_rt] = _build(n_rt)
    return _CACHE[n_rt]


def _transform(points, poses, idx):
    P = poses[idx]                                   # [N,4,4]
    R, t = P[:, :3, :3], P[:, :3, 3]
    return np.einsum('nij,nj->ni', R, points) + t    # [N,3]


def _split16(x):
    """Two-term fp16 split: x ~= hi + lo with hi = fp16(x)."""
    hi = x.astype(np.float16)
    lo = (x - hi.astype(np.float64)).astype(np.float16)
    return hi, lo


def kernel(points, time_indice, est_poses, gt_poses):
    points = np.asarray(points, dtype=np.float32)
    ti = np.asarray(time_indice)
    est_poses = np.asarray(est_poses, dtype=np.float32)
    gt_poses = np.asarray(gt_poses, dtype=np.float32)

    est = _transform(points, est_poses, ti).astype(np.float64)  # [N,3]
    gt = _transform(points, gt_poses, ti).astype(np.float64)    # [N,3]
    est_sq = np.sum(est * est, axis=1)               # [N] f64
    gt_sq = np.sum(gt * gt, axis=1)                  # [N] f64

    sel = np.flatnonzero(ti == 1)
    m = sel.size
    denom = np.float32(m) + np.float32(1e-7)
    if m == 0:
        return np.float32(0.0), np.float32(0.0)

    l2 = np.float32(
        np.linalg.norm(est[sel] - gt[sel], axis=1).sum() / denom)

    n_rt = -(-m // 128)
    mpad = n_rt * 128
    pad = np.concatenate([sel, np.repeat(sel[:1], mpad - m)])
    ncc = NSHARD // FREE

    def lhs_rows(sel_pts):
        a = -2.0 * sel_pts[pad]                      # [mpad,3] f64
        a_hi, a_lo = _split16(a)
        out = np.empty((K, mpad), np.float16)
        out[0:3] = a_hi.T
        out[3:6] = a_hi.T
        out[6:9] = a_lo.T
        out[9:11] = np.float16(1.0)
        return out

    def lhs_for(lrows):
        # row-tiles 1.. only; tile 0 is packed into the rhs tensor
        return np.ascontiguousarray(
            np.broadcast_to(lrows[:, 128:], (ncc, K, mpad - 128)))

    def rhs_for(pts, sq, c, lrows):
        s = slice(c * NSHARD, (c + 1) * NSHARD)
        p_hi, p_lo = _split16(pts[s])                # [2048,3]
        q_hi, q_lo = _split16(sq[s])                 # [2048]
        out = np.empty((K, NSHARD), np.float16)
        out[0:3] = p_hi.T
        out[3:6] = p_lo.T
        out[6:9] = p_hi.T
        out[9] = q_hi
        out[10] = q_lo
        # [K, 2048] -> [4, K, 512] chunk-major, with the tile-0 lhs slice
        # appended to each group so one DMA delivers both
        chunks = out.reshape(K, ncc, FREE).transpose(1, 0, 2)
        l0 = np.broadcast_to(lrows[:, 0:128], (ncc, K, 128))
        return np.ascontiguousarray(np.concatenate([chunks, l0], axis=2))

    lrowsA = lhs_rows(gt)   # dist1: selected gt rows vs all est points
    lrowsB = lhs_rows(est)  # dist2: selected est rows vs all gt points
    lhsA = lhs_for(lrowsA)
    lhsB = lhs_for(lrowsB)
    in_maps = [
        {
            "lhsA": lhsA,
            "rhsA": rhs_for(est, est_sq, c, lrowsA),
            "lhsB": lhsB,
            "rhsB": rhs_for(gt, gt_sq, c, lrowsB),
        }
        for c in range(N_CORES)
    ]

    nc = _get_program(n_rt)
    results = run_bass_kernel_spmd(nc, in_maps, list(range(N_CORES))).results

    # [128, n_rt] per core -> global min across cores -> flatten row-tiles
    partA = np.min([r["outA"] for r in results], axis=0).T.ravel()[:m]
    partB = np.min([r["outB"] for r in results], axis=0).T.ravel()[:m]
    dist1 = partA.astype(np.float64) + gt_sq[sel]
    dist2 = partB.astype(np.float64) + est_sq[sel]
    chamfer = np.float32(0.5 * (dist1.sum() + dist2.sum()) / denom)
    return chamfer, l2
